# revision 5
# baseline (speedup 1.0000x reference)
"""CharLSTM (2-layer, H=256, B=512, T=512) Trainium2 Bass kernel.

Strategy: data-parallel over batch across 8 cores (64 batch/core).
Per core, a software-pipelined wavefront runs layer0 step t and layer1
step t-1 concurrently. All matmuls keep weights stationary (bf16, FWL),
states/gates layout is [4H-on-partitions x batch-on-free], PSUM holds
fp32 gate pre-activations, c-state stays fp32, h-state bf16.

Weight folding done on host:
  - gate rows permuted from (i,f,g,o) to (i,f,o,g) so one sigmoid covers
    the whole 512-wide gate tile: tanh(g) is computed as 2*sigmoid(2g)-1,
    with the g-gate weight rows pre-scaled by 2 on the host. This gives
    ONE activation op per layer-step for all four gates (the Act engine
    has a ~260ns fixed cost per op, so fewer/wider ops win).
  - layer0: embedding lookup E[x] done on host (8-dim, tiny); the
    layer-0 input projection is an extra K=9 matmul chunk [Wih0.T; b0]
    against [emb_t; 1], which also folds the layer-0 bias.
  - layer1 input projection folded into the recurrent matmul as extra
    K-chunks ([Whh1|Wih1] @ [h1; h0]); bias b1 added into PSUM by one
    DVE op per step.

Scheduling: every step's instruction groups get explicit monotone
priority bands (L0(s) < emb(s+2) < L1(s-1) < L0(s+1) < ...) so the
list scheduler keeps the PE working on the critical L0 recurrence
first and fills the activation-phase gaps with L1/emb matmuls.
"""

import sys

sys.path.insert(0, "/opt/trn_rl_repo")

from contextlib import ExitStack

import numpy as np
import ml_dtypes

VOCAB = 78
EMBED = 8
H = 256
BATCH = 512
SEQ = 512
NCORES = 8
BPC = BATCH // NCORES  # 64 batch per core
EBLK = 64  # emb prefetch block (steps)

_cache = {}


def _build_program(T):
    import concourse.tile as tile
    import concourse.mybir as mybir
    from concourse import bacc

    dt = mybir.dt
    AF = mybir.ActivationFunctionType
    ALU = mybir.AluOpType
    f32, bf16 = dt.float32, dt.bfloat16

    nc = bacc.Bacc("TRN2", target_bir_lowering=False, debug=False,
                   num_devices=NCORES)

    W0h_d = nc.dram_tensor("W0h", [128, 2, 1024], bf16, kind="ExternalInput").ap()
    W0e_d = nc.dram_tensor("W0e", [9, 1024], bf16, kind="ExternalInput").ap()
    W1_d = nc.dram_tensor("W1", [128, 4, 1024], bf16, kind="ExternalInput").ap()
    B1_d = nc.dram_tensor("B1", [128, 512], f32, kind="ExternalInput").ap()
    Wfc_d = nc.dram_tensor("WfcT", [128, 2, VOCAB], bf16, kind="ExternalInput").ap()
    bfc_d = nc.dram_tensor("bfc", [VOCAB, 1], f32, kind="ExternalInput").ap()
    emb_d = nc.dram_tensor("embT", [9, T * BPC], bf16, kind="ExternalInput").ap()
    out_d = nc.dram_tensor("out", [VOCAB, BPC], f32, kind="ExternalOutput").ap()

    with tile.TileContext(nc) as tc, ExitStack() as ctx:
        const = ctx.enter_context(tc.tile_pool(name="const", bufs=1))
        W0h = const.tile([128, 2, 1024], bf16)
        nc.sync.dma_start(W0h[:], W0h_d)
        W0e = const.tile([9, 1024], bf16)
        nc.sync.dma_start(W0e[:], W0e_d)
        W1 = const.tile([128, 4, 1024], bf16)
        nc.sync.dma_start(W1[:], W1_d)
        B1 = const.tile([128, 512], f32)
        nc.sync.dma_start(B1[:], B1_d)
        WfcT = const.tile([128, 2, VOCAB], bf16)
        nc.sync.dma_start(WfcT[:], Wfc_d)
        bfc = const.tile([VOCAB, 1], f32)
        nc.sync.dma_start(bfc[:], bfc_d)

        embp = ctx.enter_context(tc.tile_pool(name="embp", bufs=2))
        ps0p = ctx.enter_context(tc.tile_pool(name="ps0p", bufs=4, space="PSUM"))
        ps1p = ctx.enter_context(tc.tile_pool(name="ps1p", bufs=2, space="PSUM"))
        psfcp = ctx.enter_context(tc.tile_pool(name="psfcp", bufs=1, space="PSUM"))
        sp = ctx.enter_context(tc.tile_pool(name="sp", bufs=2))
        s1p = ctx.enter_context(tc.tile_pool(name="s1p", bufs=2))
        pp = ctx.enter_context(tc.tile_pool(name="pp", bufs=2))
        t2p = ctx.enter_context(tc.tile_pool(name="t2p", bufs=2))
        up = ctx.enter_context(tc.tile_pool(name="up", bufs=2))
        cp = ctx.enter_context(tc.tile_pool(name="cp", bufs=2))
        hp = ctx.enter_context(tc.tile_pool(name="hp", bufs=2))
        tcp = ctx.enter_context(tc.tile_pool(name="tcp", bufs=2))
        fcp = ctx.enter_context(tc.tile_pool(name="fcp", bufs=1))

        eblk = min(EBLK, T)
        nblk = (T + eblk - 1) // eblk
        emb_tiles = [None] * nblk
        ps0_tiles = {}
        h0_prev = c0_prev = h1_prev = c1_prev = None
        pend1 = None

        def emit_emb(step):
            # layer-0 input-projection matmuls for `step`, into a fresh ps0
            # tile. One start=True per PSUM bank: start clears has_written
            # for the whole bank, so only the first MM into the bank may set
            # it; per-element has_written then handles overwrite-vs-accum
            # for every later MM (emb slices and the step-`step` h-matmuls).
            ps0 = ps0p.tile([128, 512], f32, name="ps0")
            ps0_tiles[step] = ps0
            emb_sb = emb_tiles[step // eblk]
            erhs = emb_sb[:, (step % eblk) * BPC:(step % eblk + 1) * BPC]
            for m in range(8):
                nc.tensor.matmul(ps0[:, m * 64:(m + 1) * 64],
                                 W0e[:, m * 128:(m + 1) * 128], erhs,
                                 start=(m == 0),
                                 stop=(step == 0 and m == 7),
                                 skip_group_check=True)

        def dep_nop(eng, ap):
            # In-order engine queues: a nop that reads `ap` forces every
            # later op on this engine to wait for `ap`'s producer. Used to
            # keep layer-1 work out of the layer-0 critical chain.
            n = eng.nop(hint="dep")
            n.ins.ins = [eng.lower_ap(ap)]

        def cell(s_t, c_in, pool_suffix, first):
            # s_t: [128,512] bf16 sigmoid outputs: i f o g~ blocks.
            # returns (h, c): h bf16 [128,128], c f32 [128,128]
            # g = 2*g~ - 1 (tanh via sigmoid); c = f*c_in + i*g
            #   q  = 2*g~ - 1        (tensor_scalar: 4x DVE mode, ~95ns)
            #   t2 = i * q           (= i*g)
            #   c  = f*c_in + t2     (first step: c = t2)
            i_ = s_t[:, 0:128]
            f_ = s_t[:, 128:256]
            o_ = s_t[:, 256:384]
            gs = s_t[:, 384:512]
            q = pp.tile([128, 128], bf16, name="q" + pool_suffix)
            nc.vector.tensor_scalar(q[:], gs, 2.0, -1.0, ALU.mult, ALU.add)
            c = cp.tile([128, 128], f32, name="c" + pool_suffix)
            if first:
                nc.vector.tensor_mul(c[:], i_, q[:])
            else:
                t2 = t2p.tile([128, 128], bf16, name="t2" + pool_suffix)
                nc.vector.tensor_mul(t2[:], i_, q[:])
                u = up.tile([128, 128], f32, name="u" + pool_suffix)
                nc.vector.tensor_mul(u[:], f_, c_in[:])
                nc.vector.tensor_add(c[:], u[:], t2[:])
            tc_ = tcp.tile([128, 128], bf16, name="tc" + pool_suffix)
            nc.scalar.activation(tc_[:], c[:], AF.Tanh)
            h = hp.tile([128, 128], bf16, name="h" + pool_suffix)
            nc.vector.tensor_mul(h[:], o_, tc_[:])
            return h, c, tc_

        for s in range(T + 1):
            # prefetch emb blocks: block 0 at s=0, block b+1 at start of block b
            if s < T and s % eblk == 0:
                b = s // eblk
                if b == 0:
                    e0 = embp.tile([9, eblk * BPC], bf16, name="embblk")
                    nc.sync.dma_start(e0[:], emb_d[:, 0:eblk * BPC])
                    emb_tiles[0] = e0
                if b + 1 < nblk:
                    e1 = embp.tile([9, eblk * BPC], bf16, name="embblk")
                    nc.sync.dma_start(
                        e1[:],
                        emb_d[:, (b + 1) * eblk * BPC:(b + 2) * eblk * BPC])
                    emb_tiles[b + 1] = e1

            h0_in, c0_in = h0_prev, c0_prev  # h0(s-1), c0(s-1)

            if s == 0:
                emit_emb(0)
                if T > 1:
                    emit_emb(1)

            if s < T:
                # ---- layer 0, step s: the critical recurrence cycle ----
                tc.cur_priority = s * 1000
                ps0 = ps0_tiles.pop(s)
                if s > 0:
                    for m in range(8):
                        o = ps0[:, m * 64:(m + 1) * 64]
                        for k in range(2):
                            nc.tensor.matmul(
                                o, W0h[:, k, m * 128:(m + 1) * 128],
                                h0_in[:, k * 64:(k + 1) * 64],
                                start=False, stop=(m == 7 and k == 1),
                                skip_group_check=True)
                s0 = sp.tile([128, 512], bf16, name="s0")
                nc.scalar.activation(s0[:], ps0[:], AF.Sigmoid)
                h0, c0, tc0 = cell(s0, c0_in, "0", s == 0)
                h0_prev, c0_prev = h0, c0

            # ---- layer 1, step s-2: cell phase (deferred one cycle so its
            # tanh/DVE ops can be queued strictly behind this cycle's
            # layer-0 critical chain; otherwise the in-order Act/DVE queues
            # stall the next sigmoid on layer-1's slow cell ops) ----
            if pend1 is not None:
                tc.cur_priority = s * 1000 + 500
                s1_p, c1_in_p, first_p = pend1
                if s < T:
                    dep_nop(nc.vector, h0_prev[:])
                    dep_nop(nc.scalar, tc0[:])
                h1, c1, _ = cell(s1_p, c1_in_p, "1", first_p)
                h1_prev, c1_prev = h1, c1
                pend1 = None

            if s + 2 < T:
                tc.cur_priority = s * 1000 + 400
                emit_emb(s + 2)

            if s > 0:
                # ---- layer 1, step s-1: matmul + sigmoid phase ----
                tc.cur_priority = s * 1000 + 600
                ps1 = ps1p.tile([128, 512], f32)
                for m in range(8):
                    o = ps1[:, m * 64:(m + 1) * 64]
                    if s > 1:
                        for k in range(2):
                            nc.tensor.matmul(
                                o, W1[:, k, m * 128:(m + 1) * 128],
                                h1_prev[:, k * 64:(k + 1) * 64],
                                start=(k == 0), stop=False)
                    for k in range(2):
                        nc.tensor.matmul(
                            o, W1[:, 2 + k, m * 128:(m + 1) * 128],
                            h0_in[:, k * 64:(k + 1) * 64],
                            start=(s == 1 and k == 0), stop=(k == 1))
                badd = nc.vector.tensor_add(ps1[:], ps1[:], B1[:])
                if s < T:
                    # keep the B1 bias-add (which waits on the whole ps1 MM
                    # block) from being queue-ordered before this step's
                    # critical h0-mul on the in-order DVE queue: give it a
                    # real read-dep on h0 so the list scheduler must place it
                    # after the h0-mul
                    badd.ins.ins = badd.ins.ins + [nc.vector.lower_ap(h0_prev[:])]
                if s < T:
                    # sigmoid_B may not preempt this cycle's tanh(c0)
                    dep_nop(nc.scalar, tc0[:])
                s1 = s1p.tile([128, 512], bf16, name="s1")
                nc.scalar.activation(s1[:], ps1[:], AF.Sigmoid)
                pend1 = (s1, c1_prev, s == 1)

        # ---- drain the last pending layer-1 cell (step T-1) ----
        if pend1 is not None:
            tc.cur_priority = (T + 1) * 1000
            s1_p, c1_in_p, first_p = pend1
            h1, c1, _ = cell(s1_p, c1_in_p, "1", first_p)
            h1_prev, c1_prev = h1, c1
            pend1 = None

        # ---- final FC on h1(T-1) ----
        tc.cur_priority = (T + 2) * 1000
        psfc = psfcp.tile([VOCAB, BPC], f32)
        for k in range(2):
            nc.tensor.matmul(psfc[:], WfcT[:, k, :],
                             h1_prev[:, k * 64:(k + 1) * 64],
                             start=(k == 0), stop=(k == 1))
        fc = fcp.tile([VOCAB, BPC], f32)
        nc.scalar.activation(fc[:], psfc[:], AF.Identity, bias=bfc[:])
        nc.sync.dma_start(out_d, fc[:])

    nc.compile()
    return nc


def _prep_inputs(x, E, Wih0, Whh0, bih0, bhh0, Wih1, Whh1, bih1, bhh1,
                 Wfc, bfc, T):
    """Host-side weight folding and per-core input shards."""
    bf16 = ml_dtypes.bfloat16
    # permute gate rows (i,f,g,o) -> (i,f,o,g)
    perm = np.r_[0:256, 256:512, 768:1024, 512:768]
    Wih0 = np.asarray(Wih0, np.float32)[perm]
    Whh0 = np.asarray(Whh0, np.float32)[perm]
    b0 = (np.asarray(bih0, np.float32) + np.asarray(bhh0, np.float32))[perm]
    Wih1 = np.asarray(Wih1, np.float32)[perm]
    Whh1 = np.asarray(Whh1, np.float32)[perm]
    b1 = (np.asarray(bih1, np.float32) + np.asarray(bhh1, np.float32))[perm]
    # tanh(g) = 2*sigmoid(2g) - 1: fold the 2x into the g-gate rows
    for M in (Wih0, Whh0, b0, Wih1, Whh1, b1):
        M[768:1024] *= 2.0
    Wfc = np.asarray(Wfc, np.float32)
    bfc = np.asarray(bfc, np.float32)

    W0h = np.ascontiguousarray(
        Whh0.T.reshape(2, 128, 1024).transpose(1, 0, 2)).astype(bf16)
    W0e = np.concatenate([Wih0.T, b0[None, :]], axis=0).astype(bf16)  # [9,1024]
    W1 = np.ascontiguousarray(
        np.concatenate([Whh1.T, Wih1.T], axis=0)  # [512, 1024]
        .reshape(4, 128, 1024).transpose(1, 0, 2)).astype(bf16)
    B1 = np.ascontiguousarray(
        np.broadcast_to(b1.reshape(8, 128).T[:, :, None],
                        (128, 8, 64)).reshape(128, 512)).astype(np.float32)
    WfcT = np.ascontiguousarray(
        Wfc.T.reshape(2, 128, VOCAB).transpose(1, 0, 2)).astype(bf16)
    bfc2 = np.ascontiguousarray(bfc[:, None]).astype(np.float32)

    E2 = np.asarray(E, np.float32).copy()
    E2[0] = 0.0  # padding_idx=0
    x = np.asarray(x)

    common = {"W0h": W0h, "W0e": W0e, "W1": W1, "B1": B1, "WfcT": WfcT,
              "bfc": bfc2}
    in_maps = []
    for i in range(NCORES):
        xs = x[i * BPC:(i + 1) * BPC, :T]  # [64, T]
        emb = E2[xs]  # [64, T, 8]
        embT = np.empty((9, T, BPC), np.float32)
        embT[:8] = emb.transpose(2, 1, 0)
        embT[8] = 1.0
        m = dict(common)
        m["embT"] = np.ascontiguousarray(embT.reshape(9, T * BPC)).astype(bf16)
        in_maps.append(m)
    return in_maps


def kernel(x, E, Wih0, Whh0, bih0, bhh0, Wih1, Whh1, bih1, bhh1, Wfc, bfc,
           T=SEQ, trace=False):
    from concourse import bass_utils

    if T not in _cache:
        _cache[T] = _build_program(T)
    nc = _cache[T]
    in_maps = _prep_inputs(x, E, Wih0, Whh0, bih0, bhh0, Wih1, Whh1, bih1,
                           bhh1, Wfc, bfc, T)
    res = bass_utils.run_bass_kernel_spmd(nc, in_maps, list(range(NCORES)),
                                          trace=trace)
    out = np.empty((BATCH, VOCAB), np.float32)
    for i in range(NCORES):
        out[i * BPC:(i + 1) * BPC] = np.asarray(res.results[i]["out"]).T
    if trace:
        return out, res
    return out

